# revision 1
# baseline (speedup 1.0000x reference)
"""Multi-head attention (b=2, l=2048, d_model=1024, h=16) on 8 trn2 NeuronCores.

Sharding: tensor-parallel over heads. Each core owns 2 heads: it computes the
QKV projections for its 128 channels (transposed layout), attention for its
heads, and a rank-128 partial of the output projection. The host sums the 8
partials and adds b_o (the tensor-parallel all-reduce, done at gather time).

On-device layout/algorithm per core (all matmuls in float32r, fp32 accumulate):
  warmup:  identity matmul burst to lift the PE HAM clock gate + a dummy exp
           to preload the ACT spline table while input DMAs run.
  phase A: QT/KT/VT [128ch, 4096tok] = W.T @ xT, streamed over 512-token
           chunks; V transposed back to natural [tok, ch] tiles via PE
           transpose, augmented with a ones column (for softmax sums).
  phase B: per (batch, 1024-q-chunk, k-tile): both heads' scoresT[k,q] =
           KT_h'-slice @ QT_h-slice back to back (disjoint PE row groups run
           concurrently); expT = exp(scoresT) on ACT (the phase pacer);
           PV accumulates [V_h | 1].T @ expT into psum [65, 1024] =
           unnormalized attnT plus softmax denominators Z.
  phase C: partial_out[tok, :] = sum_h (attnU_h.T @ Wo_h) * (1/Z_h per token);
           h0/h1 matmuls pair on PE row groups; normalization deferred to
           per-partition scales (ACT) + fused multiply-add (DVE) at PSUM
           evacuation. 1/sqrt(dh) is folded into Wq/bq on the host.
"""
import sys
import types

import numpy as np

D_MODEL = 1024
H = 16
DH = 64
B = 2
L = 2048
BL = B * L            # 4096 tokens
NCORES = 8
NKT = D_MODEL // 128  # 8 feature tiles
TCH = 512             # phase-A token chunk
NCH = BL // TCH       # 8 chunks
QC = 1024             # phase-B q chunk
NQC = L // QC         # 2 per batch
NKB = L // 128        # 16 k-tiles per batch
VSTRIDE = 2 * (DH + 1)  # per-k-tile Vaug columns: [V_h0 | 1 | V_h1 | 1]


def _register_ntff_hook():
    """Install the axon NTFF profiling hook module if the image lacks it.

    Harmless if never used; required for run_bass_kernel_spmd(trace=True)."""
    if "antenv.axon_hooks" in sys.modules:
        return
    try:
        import antenv
        mod = types.ModuleType("antenv.axon_hooks")
        holder = {}
        mod.set_axon_ntff_profile_hook = lambda h: holder.__setitem__("h", h)
        mod.get_axon_ntff_profile_hook = lambda: holder.get("h")
        sys.modules["antenv.axon_hooks"] = mod
        antenv.axon_hooks = mod
        from trn_agent_boot.trn_boot import _ntff_profile_via_ctypes
        mod.set_axon_ntff_profile_hook(
            _ntff_profile_via_ctypes("/opt/axon/libaxon_pjrt.so")
        )
    except Exception:
        pass


_NC_CACHE = {}


def _build():
    if "nc" in _NC_CACHE:
        return _NC_CACHE["nc"]
    import concourse.bacc as bacc
    import concourse.tile as tile
    import concourse.mybir as mybir

    F32 = mybir.dt.float32
    F32R = mybir.dt.float32r
    AF = mybir.ActivationFunctionType
    ALU = mybir.AluOpType

    nc = bacc.Bacc("TRN2", target_bir_lowering=False, debug=False)

    xT_d = nc.dram_tensor("xT", [D_MODEL, BL], F32R, kind="ExternalInput").ap()
    wq_d = nc.dram_tensor("wq", [128, NKT * 128], F32R, kind="ExternalInput").ap()
    wk_d = nc.dram_tensor("wk", [128, NKT * 128], F32R, kind="ExternalInput").ap()
    wv_d = nc.dram_tensor("wv", [128, NKT * 128], F32R, kind="ExternalInput").ap()
    bq_d = nc.dram_tensor("bq", [128, 1], F32, kind="ExternalInput").ap()
    bk_d = nc.dram_tensor("bk", [128, 1], F32, kind="ExternalInput").ap()
    bv_d = nc.dram_tensor("bv", [128, 1], F32, kind="ExternalInput").ap()
    wo_d = nc.dram_tensor("wo", [128, D_MODEL], F32R, kind="ExternalInput").ap()
    id_d = nc.dram_tensor("ident", [128, 128], F32R, kind="ExternalInput").ap()
    out_d = nc.dram_tensor("out", [BL, D_MODEL], F32, kind="ExternalOutput").ap()

    with tile.TileContext(nc) as tc:
        with (
            tc.tile_pool(name="weights", bufs=1) as wpool,
            tc.tile_pool(name="persist", bufs=1) as ppool,
        ):
            id_t = wpool.tile([128, 128], F32R, tag="ident")
            nc.gpsimd.dma_start(id_t[:], id_d)
            wq_t = wpool.tile([128, NKT * 128], F32R, tag="wq")
            wk_t = wpool.tile([128, NKT * 128], F32R, tag="wk")
            wv_t = wpool.tile([128, NKT * 128], F32R, tag="wv")
            bq_t = wpool.tile([128, 1], F32, tag="bq")
            bk_t = wpool.tile([128, 1], F32, tag="bk")
            bv_t = wpool.tile([128, 1], F32, tag="bv")
            wo_t = wpool.tile([128, D_MODEL], F32R, tag="wo")
            for t, d in ((wq_t, wq_d), (wk_t, wk_d), (wv_t, wv_d),
                         (bq_t, bq_d), (bk_t, bk_d), (bv_t, bv_d),
                         (wo_t, wo_d)):
                nc.gpsimd.dma_start(t[:], d)

            QT = ppool.tile([128, BL], F32R, tag="QT")
            KT = ppool.tile([128, BL], F32R, tag="KT")
            VT = ppool.tile([128, BL], F32R, tag="VT")
            Vaug = ppool.tile([128, (BL // 128) * VSTRIDE], F32R, tag="Vaug")
            attnU = [ppool.tile([128, L], F32R, tag=f"attnU{b}",
                                name=f"attnU{b}") for b in range(B)]
            zrow = [[ppool.tile([1, L], F32, tag=f"zrow{h}{b}",
                                name=f"zrow{h}{b}") for b in range(B)]
                    for h in range(2)]
            rz = [[ppool.tile([128, L // 128], F32, tag=f"rz{h}{b}",
                              name=f"rz{h}{b}") for b in range(B)]
                  for h in range(2)]
            scr = ppool.tile([1, 32], F32, tag="scr")

            nc.vector.memset(Vaug[:].bitcast(F32), 1.0)

            # ---- warmup: lift HAM clock gate + preload exp table ----
            with tc.tile_pool(name="psW", bufs=1, space="PSUM") as psW:
                wu = psW.tile([128, 512], F32, tag="wu")
                for i in range(40):
                    nc.tensor.matmul(wu[:, 0:128], id_t[:], id_t[:],
                                     start=(i == 0), stop=(i == 39))
                nc.scalar.activation(scr[:], wu[0:1, 0:32], AF.Exp)

            # ---- phase A: QKV projections (transposed) + V re-transpose ----
            with (
                tc.tile_pool(name="xin", bufs=2) as xpool,
                tc.tile_pool(name="psA", bufs=4, space="PSUM") as psA,
                tc.tile_pool(name="psT", bufs=2, space="PSUM") as psT,
            ):
                for c in range(NCH):
                    sl = slice(c * TCH, (c + 1) * TCH)
                    xt = xpool.tile([128, NKT, TCH], F32R, tag="xchunk")
                    for kt in range(NKT):
                        nc.sync.dma_start(
                            xt[:, kt, :], xT_d[kt * 128:(kt + 1) * 128, sl]
                        )
                    for w_t, b_t, dst in ((wq_t, bq_t, QT), (wk_t, bk_t, KT),
                                          (wv_t, bv_t, VT)):
                        ps = psA.tile([128, TCH], F32, tag="projps")
                        for kt in range(NKT):
                            nc.tensor.matmul(
                                ps[:], w_t[:, kt * 128:(kt + 1) * 128],
                                xt[:, kt, :],
                                start=(kt == 0), stop=(kt == NKT - 1),
                            )
                        nc.vector.tensor_scalar_add(dst[:, sl], ps[:], b_t[:, 0:1])
                    # natural-layout V for the k-tiles this chunk completed
                    for g in range(c * (TCH // 128), (c + 1) * (TCH // 128)):
                        tp = psT.tile([128, 128], F32R, tag="vtrans")
                        nc.tensor.transpose(
                            tp[:], VT[:, g * 128:(g + 1) * 128], id_t[:]
                        )
                        base = g * VSTRIDE
                        nc.vector.tensor_copy(
                            Vaug[:, base:base + DH], tp[:, 0:DH]
                        )
                        nc.vector.tensor_copy(
                            Vaug[:, base + DH + 1:base + 2 * DH + 1],
                            tp[:, DH:2 * DH],
                        )

            # ---- phase B: scoresT -> exp -> PV (ACT paces; PE kept dense) ----
            # One head at a time; scores double-buffered so the PE's runnable
            # window stays deep (enables LDWEIGHTS pull-ahead). A dedicated
            # filler bank takes dep-free full-array matmuls each k-tile so the
            # PE never shows the HAM clock gate an idle window.
            with (
                tc.tile_pool(name="expP", bufs=3) as epool,
                tc.tile_pool(name="att65P", bufs=4) as apool,
                tc.tile_pool(name="oout", bufs=3) as opool,
                tc.tile_pool(name="dram", bufs=1, space="DRAM") as dpool,
                tc.tile_pool(name="psS", bufs=2, space="PSUM") as psS,
                tc.tile_pool(name="psPV", bufs=1, space="PSUM") as psPV,
                tc.tile_pool(name="psO", bufs=1, space="PSUM") as psO,
            ):
                zscr = dpool.tile([2, BL], F32, tag="zscr")

                def emit_filler(pool, tag):
                    """Dep-free full-array matmul: keeps the HAM clock gate
                    open during ACT/evac-paced stretches."""
                    f = pool.tile([128, 512], F32, tag=tag, name="fillt")
                    nc.tensor.matmul(f[:, 0:384], id_t[:], QT[:, 0:384],
                                     start=True, stop=True)

                def emit_c_unit(rc, oc, tail, alt=False):
                    """One output-projection unit: [128 tok, 512] both heads,
                    normalized via deferred per-partition 1/Z scales.

                    During the overlap with attention (tail=False) ACT is busy
                    with exps, so both evacuation ops go to DVE. In the tail,
                    psum tiles alternate into the idle scores/PV slots for
                    pipeline depth, and fillers keep the PE clock gate open."""
                    rsl = slice(rc * 128, (rc + 1) * 128)
                    bi, lrc = rc // (L // 128), rc % (L // 128)
                    lrsl = slice(lrc * 128, (lrc + 1) * 128)
                    osl = slice(oc * 512, (oc + 1) * 512)
                    if alt:
                        ps0 = psS.tile([128, 512], F32, tag="sc", name="ps0a")
                        ps1 = psS.tile([128, 512], F32, tag="sc", name="ps1a")
                    else:
                        ps0 = psO.tile([128, 512], F32, tag="ps0", name="ps0")
                        ps1 = psO.tile([128, 512], F32, tag="ps1", name="ps1")
                    # adjacent pair: row groups 0-63 / 64-127 overlap on PE
                    nc.tensor.matmul(ps0[:], attnU[bi][0:64, lrsl],
                                     wo_t[0:64, osl], start=True, stop=True)
                    nc.tensor.matmul(ps1[:], attnU[bi][64:128, lrsl],
                                     wo_t[64:128, osl], start=True, stop=True)
                    if tail:
                        emit_filler(psPV, "pv")
                    tmp = opool.tile([128, 512], F32, tag="tmp", name="tmp")
                    if tail:
                        nc.scalar.activation(tmp[:], ps0[:], AF.Copy,
                                             scale=rz[0][bi][:, lrc:lrc + 1])
                    else:
                        nc.vector.tensor_scalar_mul(tmp[:], ps0[:],
                                                    rz[0][bi][:, lrc:lrc + 1])
                    ot = opool.tile([128, 512], F32, tag="ot", name="ot")
                    nc.vector.scalar_tensor_tensor(
                        ot[:], ps1[:], rz[1][bi][:, lrc:lrc + 1], tmp[:],
                        op0=ALU.mult, op1=ALU.add,
                    )
                    nc.sync.dma_start(out_d[rsl, osl], ot[:])

                # Output-projection units become PE keep-warm work inside the
                # ACT-paced attention stretches as soon as their inputs exist:
                # batch-0 units during (b1,h0) + (b1,h1,qc0); batch-1's first
                # half during (b1,h1,qc1); only the last 16 run in the tail.
                c_queue = []
                budget = 0.0
                # dense bridge over the phase-A -> B transition: never show
                # the HAM clock gate a low-activity window
                for _ in range(16):
                    emit_filler(psO, "ps0")
                for b in range(B):
                    for h in range(2):
                        hs = slice(h * 64, (h + 1) * 64)
                        for qc in range(NQC):
                            if b == 1 and h == 0 and qc == 0:
                                c_queue += [(rc, oc) for rc in range(16)
                                            for oc in range(2)]
                            if b == 1 and h == 1 and qc == 1:
                                c_queue += [(rc, oc) for rc in range(16, 24)
                                            for oc in range(2)]
                            rate = 1.0
                            q0 = b * L + qc * QC
                            qsl = slice(q0, q0 + QC)
                            pv = psPV.tile([65, QC], F32, tag="pv")
                            for kt in range(NKB):
                                ksl = slice(b * L + kt * 128,
                                            b * L + (kt + 1) * 128)
                                sc = psS.tile([128, QC], F32, tag="sc")
                                for hf in range(QC // 512):
                                    nc.tensor.matmul(
                                        sc[:, hf * 512:(hf + 1) * 512],
                                        KT[hs, ksl],
                                        QT[hs, q0 + hf * 512:q0 + hf * 512 + 512],
                                        start=True, stop=True,
                                    )
                                ex = epool.tile([128, QC], F32R, tag="ex")
                                nc.scalar.activation(ex[:], sc[:], AF.Exp)
                                g = b * NKB + kt
                                vb = g * VSTRIDE + h * (DH + 1)
                                for hf in range(QC // 512):
                                    nc.tensor.matmul(
                                        pv[:, hf * 512:(hf + 1) * 512],
                                        Vaug[:, vb:vb + DH + 1],
                                        ex[:, hf * 512:(hf + 1) * 512],
                                        start=(kt == 0), stop=(kt == NKB - 1),
                                    )
                                if c_queue:
                                    budget += rate
                                    if budget >= 1.0:
                                        budget -= 1.0
                                        emit_c_unit(*c_queue.pop(0), tail=False)
                                    else:
                                        emit_filler(psO, "ps0")
                                else:
                                    emit_filler(psO, "ps0")
                            # bridge the evacuation bubble at the chunk edge
                            for _ in range(8):
                                emit_filler(psO, "ps0")
                            # evacuate: one copy frees the accumulator; the
                            # attnU/Z split happens off the critical path
                            a65 = apool.tile([65, QC], F32R, tag="a65")
                            nc.vector.tensor_copy(a65[:], pv[0:65, :])
                            lqsl = slice(qc * QC, (qc + 1) * QC)
                            nc.vector.tensor_copy(
                                attnU[b][h * 64:(h + 1) * 64, lqsl], a65[0:64, :]
                            )
                            nc.vector.tensor_copy(zrow[h][b][:, lqsl],
                                                  a65[64:65, :])
                            # softmax denominators -> reciprocal columns via
                            # DRAM bounce; per q-chunk on the final stretch so
                            # its output projection can start early
                            zparts = ([lqsl] if (b == 1 and h == 1) or
                                      qc == NQC - 1 else [])
                            if b != 1 or h != 1:
                                zparts = ([slice(0, L)] if qc == NQC - 1 else [])
                            for zsl in zparts:
                                nc.sync.dma_start(
                                    zscr[h:h + 1, b * L + zsl.start:
                                         b * L + zsl.stop],
                                    zrow[h][b][:, zsl])
                                zc = ppool.tile(
                                    [128, (zsl.stop - zsl.start) // 128], F32,
                                    tag=f"zc{h}{b}{qc}", name=f"zc{h}{b}{qc}")
                                nc.sync.dma_start(
                                    zc[:],
                                    zscr[h, b * L + zsl.start:b * L + zsl.stop]
                                    .rearrange("(c p) -> p c", p=128),
                                )
                                nc.vector.reciprocal(
                                    rz[h][b][:, zsl.start // 128:
                                             zsl.stop // 128],
                                    zc[:],
                                )

                # leftover queued units, then the final batch-1 quarter
                c_tail = c_queue + [(rc, oc) for rc in range(24, BL // 128)
                                    for oc in range(2)]
                for i, u in enumerate(c_tail):
                    emit_c_unit(*u, tail=True, alt=(i % 2 == 1))

    nc.compile()
    _NC_CACHE["nc"] = nc
    return nc


def _shard_inputs(x, W_qkv, b_qkv, W_o):
    xT = np.ascontiguousarray(
        x.reshape(BL, D_MODEL).T, dtype=np.float32
    )
    ident = np.eye(128, dtype=np.float32)

    def lhsT_layout(w):
        # [D_MODEL, 128] -> [128, NKT*128] with [p, kt*128+ch] = w[kt*128+p, ch]
        return np.ascontiguousarray(
            w.reshape(NKT, 128, 128).transpose(1, 0, 2).reshape(128, NKT * 128),
            dtype=np.float32,
        )

    in_maps = []
    for c in range(NCORES):
        cs = slice(c * 128, (c + 1) * 128)
        wq = W_qkv[:, cs] * 0.125
        wk = W_qkv[:, D_MODEL:][:, cs]
        wv = W_qkv[:, 2 * D_MODEL:][:, cs]
        in_maps.append({
            "xT": xT,
            "wq": lhsT_layout(wq), "wk": lhsT_layout(wk), "wv": lhsT_layout(wv),
            "bq": np.ascontiguousarray(
                b_qkv[cs] * 0.125, dtype=np.float32).reshape(128, 1),
            "bk": np.ascontiguousarray(
                b_qkv[D_MODEL:][cs], dtype=np.float32).reshape(128, 1),
            "bv": np.ascontiguousarray(
                b_qkv[2 * D_MODEL:][cs], dtype=np.float32).reshape(128, 1),
            "wo": np.ascontiguousarray(W_o[cs, :], dtype=np.float32),
            "ident": ident,
        })
    return in_maps


def _run(inputs, trace=False, tmpdir=None):
    from concourse.bass_utils import run_bass_kernel_spmd

    _register_ntff_hook()
    nc = _build()
    in_maps = _shard_inputs(
        np.asarray(inputs["x"], dtype=np.float32),
        np.asarray(inputs["W_qkv"], dtype=np.float32),
        np.asarray(inputs["b_qkv"], dtype=np.float32),
        np.asarray(inputs["W_o"], dtype=np.float32),
    )
    res = run_bass_kernel_spmd(nc, in_maps, core_ids=list(range(NCORES)),
                               trace=trace, tmpdir=tmpdir)
    partial = np.zeros((BL, D_MODEL), dtype=np.float64)
    for c in range(NCORES):
        partial += res.results[c]["out"].astype(np.float64)
    out = (partial + np.asarray(inputs["b_o"], dtype=np.float64)).astype(np.float32)
    return out.reshape(B, L, D_MODEL), res


def kernel(**inputs) -> np.ndarray:
    out, _ = _run(inputs, trace=False)
    return out



# revision 8
# speedup vs baseline: 1.0953x; 1.0953x over previous
"""Multi-head attention (b=2, l=2048, d_model=1024, h=16) on 8 trn2 NeuronCores.

Sharding: tensor-parallel over heads. Each core owns 2 heads: it computes the
QKV projections for its 128 channels (transposed layout), attention for its
heads, and a rank-128 partial of the output projection. The host sums the 8
partials and adds b_o (the tensor-parallel all-reduce, done at gather time).

All matmul operands are bf16 (fp32 PSUM accumulate); numeric headroom vs the
2e-2 gate is ample. Per-core schedule is a single software-pipelined stream:

  warmup:  short identity-matmul burst (PE p-state) + exp-table preload,
           overlapped with the weight/x DMAs.
  proj:    QT/KT/VT [128ch, tok] = W.T @ xT per 512-token chunk; PSUM is
           evacuated by ACT (Copy + per-channel bias) so DVE stays free; V is
           re-transposed to key-major Vaug tiles augmented with a ones column
           (softmax denominators come out of the PV matmul's 65th row).
  attn:    per (batch, head, 1024-query chunk) unit: 16 k-tiles of
           scoresT = KT_h'@QT_h -> exp on ACT -> PV accumulate [65, 1024].
           ACT is the steady-state pacer; projection chunks for batch 1 and
           output-projection units are interleaved into the PE's slack.
  fin:     per unit: evacuate PV psum, reciprocal of the Z row, broadcast
           1/Z across partitions via a tiny ones-matmul, and scale attnU ->
           attnN (pre-normalized, bf16). This removes all normalization work
           from the output projection.
  out:     out[128tok, 1024] = attnN[:, tok-slice].T @ Wo as single
           full-128-contraction matmuls (both heads fused), DVE/ACT psum
           evacuation, one 256KB DMA per 128-token row block.
"""
import sys
import types

import numpy as np

D_MODEL = 1024
H = 16
DH = 64
B = 2
L = 2048
BL = B * L            # 4096 tokens
NCORES = 8
NKT = D_MODEL // 128  # 8 feature tiles
TCH = 512             # projection token chunk
NCH = BL // TCH       # 8 chunks
QC = 1024             # attention query chunk
NQC = L // QC         # 2 per batch
NKB = L // 128        # 16 k-tiles per batch
VSTRIDE = 2 * (DH + 1)  # per-k-tile Vaug columns: [V_h0 | 1 | V_h1 | 1]


def _register_ntff_hook():
    """Install the axon NTFF profiling hook module if the image lacks it.

    Harmless if never used; required for run_bass_kernel_spmd(trace=True)."""
    if "antenv.axon_hooks" in sys.modules:
        return
    try:
        import antenv
        mod = types.ModuleType("antenv.axon_hooks")
        holder = {}
        mod.set_axon_ntff_profile_hook = lambda h: holder.__setitem__("h", h)
        mod.get_axon_ntff_profile_hook = lambda: holder.get("h")
        sys.modules["antenv.axon_hooks"] = mod
        antenv.axon_hooks = mod
        from trn_agent_boot.trn_boot import _ntff_profile_via_ctypes
        mod.set_axon_ntff_profile_hook(
            _ntff_profile_via_ctypes("/opt/axon/libaxon_pjrt.so")
        )
    except Exception:
        pass


_NC_CACHE = {}


def _build():
    if "nc" in _NC_CACHE:
        return _NC_CACHE["nc"]
    import concourse.bacc as bacc
    import concourse.tile as tile
    import concourse.mybir as mybir

    F32 = mybir.dt.float32
    F32R = mybir.dt.float32r
    BF16 = mybir.dt.bfloat16
    AF = mybir.ActivationFunctionType
    ALU = mybir.AluOpType

    nc = bacc.Bacc("TRN2", target_bir_lowering=False, debug=False)

    xT_d = nc.dram_tensor("xT", [D_MODEL, BL], BF16, kind="ExternalInput").ap()
    wq_d = nc.dram_tensor("wq", [128, NKT * 128], BF16, kind="ExternalInput").ap()
    wk_d = nc.dram_tensor("wk", [128, NKT * 128], BF16, kind="ExternalInput").ap()
    wv_d = nc.dram_tensor("wv", [128, NKT * 128], BF16, kind="ExternalInput").ap()
    bq_d = nc.dram_tensor("bq", [128, 1], F32, kind="ExternalInput").ap()
    bk_d = nc.dram_tensor("bk", [128, 1], F32, kind="ExternalInput").ap()
    bv_d = nc.dram_tensor("bv", [128, 1], F32, kind="ExternalInput").ap()
    wo_d = nc.dram_tensor("wo", [128, D_MODEL], BF16, kind="ExternalInput").ap()
    id_d = nc.dram_tensor("ident", [128, 128], BF16, kind="ExternalInput").ap()
    out_d = nc.dram_tensor("out", [BL, D_MODEL], BF16, kind="ExternalOutput").ap()

    with tile.TileContext(nc) as tc:
        with (
            tc.tile_pool(name="weights", bufs=1) as wpool,
            tc.tile_pool(name="persist", bufs=1) as ppool,
            tc.tile_pool(name="xin", bufs=2) as xpool,
            tc.tile_pool(name="expP", bufs=3) as epool,
            tc.tile_pool(name="a65P", bufs=2) as apool,
            tc.tile_pool(name="rzP", bufs=2) as rpool,
            tc.tile_pool(name="oout", bufs=3) as opool,
            tc.tile_pool(name="psA", bufs=2, space="PSUM") as psA,
            tc.tile_pool(name="psS", bufs=2, space="PSUM") as psS,
            tc.tile_pool(name="psPV", bufs=1, space="PSUM") as psPV,
        ):
            id_t = wpool.tile([128, 128], BF16, tag="ident")
            nc.gpsimd.dma_start(id_t[:], id_d)
            wq_t = wpool.tile([128, NKT * 128], BF16, tag="wq")
            wk_t = wpool.tile([128, NKT * 128], BF16, tag="wk")
            wv_t = wpool.tile([128, NKT * 128], BF16, tag="wv")
            bq_t = wpool.tile([128, 1], F32, tag="bq")
            bk_t = wpool.tile([128, 1], F32, tag="bk")
            bv_t = wpool.tile([128, 1], F32, tag="bv")
            wo_t = wpool.tile([128, D_MODEL], BF16, tag="wo")
            for t, d in ((wq_t, wq_d), (wk_t, wk_d), (wv_t, wv_d),
                         (bq_t, bq_d), (bk_t, bk_d), (bv_t, bv_d),
                         (wo_t, wo_d)):
                nc.gpsimd.dma_start(t[:], d)

            QT = ppool.tile([128, BL], BF16, tag="QT")
            KT = ppool.tile([128, BL], BF16, tag="KT")
            VT = ppool.tile([128, BL], BF16, tag="VT")
            Vaug = ppool.tile([128, (BL // 128) * VSTRIDE], BF16, tag="Vaug")
            attnN = [ppool.tile([128, L], BF16, tag=f"attnN{b}",
                                name=f"attnN{b}") for b in range(B)]
            ones_t = ppool.tile([1, DH], F32R, tag="ones")
            scr = ppool.tile([1, 32], F32, tag="scr")

            nc.vector.memset(Vaug[:], 1.0)
            nc.vector.memset(ones_t[:].bitcast(F32), 1.0)

            # ---- warmup: PE p-state ramp + exp table preload ----
            def emit_warmup():
                wu = psS.tile([128, QC], F32, tag="sc", name="warm")
                for i in range(24):
                    nc.tensor.matmul(wu[:, 0:128], id_t[:], id_t[:],
                                     start=(i == 0), stop=(i == 23))
                nc.scalar.activation(scr[:], wu[0:1, 0:32], AF.Exp)

            # ---- one 512-token projection chunk (+ V re-transpose) ----
            def emit_chunk(c):
                sl = slice(c * TCH, (c + 1) * TCH)
                xt = xpool.tile([128, NKT, TCH], BF16, tag="xchunk")
                for kt in range(NKT):
                    eng = nc.sync if kt % 2 == 0 else nc.gpsimd
                    eng.dma_start(
                        xt[:, kt, :], xT_d[kt * 128:(kt + 1) * 128, sl]
                    )
                for w_t, b_t, dst in ((wq_t, bq_t, QT), (wk_t, bk_t, KT),
                                      (wv_t, bv_t, VT)):
                    ps = psA.tile([128, TCH], F32, tag="pA")
                    for kt in range(NKT):
                        nc.tensor.matmul(
                            ps[:], w_t[:, kt * 128:(kt + 1) * 128],
                            xt[:, kt, :],
                            start=(kt == 0), stop=(kt == NKT - 1),
                        )
                    # ACT evacuation: identity + per-channel bias (exp table
                    # has identity resident, so no table thrash with the exps)
                    nc.scalar.activation(dst[:, sl], ps[:], AF.Identity,
                                         bias=b_t[:, 0:1])
                tp = psA.tile([128, TCH], BF16, tag="pA", name="tp")
                for g4 in range(TCH // 128):
                    g = c * (TCH // 128) + g4
                    nc.tensor.transpose(
                        tp[:, g4 * 128:(g4 + 1) * 128],
                        VT[:, g * 128:(g + 1) * 128], id_t[:]
                    )
                for g4 in range(TCH // 128):
                    g = c * (TCH // 128) + g4
                    base = g * VSTRIDE
                    c0 = g4 * 128
                    nc.vector.tensor_copy(
                        Vaug[:, base:base + DH], tp[:, c0:c0 + DH]
                    )
                    nc.vector.tensor_copy(
                        Vaug[:, base + DH + 1:base + 2 * DH + 1],
                        tp[:, c0 + DH:c0 + 2 * DH],
                    )

            # ---- attention unit state ----
            unit_pv = {}

            def emit_kt(b, h, qc, kt):
                hs = slice(h * 64, (h + 1) * 64)
                q0 = b * L + qc * QC
                if kt == 0:
                    unit_pv[(b, h, qc)] = psPV.tile([65, QC], F32, tag="pv", name="pv")
                pv = unit_pv[(b, h, qc)]
                ksl = slice(b * L + kt * 128, b * L + (kt + 1) * 128)
                sc = psS.tile([128, QC], F32, tag="sc")
                for hf in range(QC // 512):
                    nc.tensor.matmul(
                        sc[:, hf * 512:(hf + 1) * 512],
                        KT[hs, ksl],
                        QT[hs, q0 + hf * 512:q0 + hf * 512 + 512],
                        start=True, stop=True,
                    )
                ex = epool.tile([128, QC], BF16, tag="ex")
                nc.scalar.activation(ex[:], sc[:], AF.Exp)
                g = b * NKB + kt
                vb = g * VSTRIDE + h * (DH + 1)
                for hf in range(QC // 512):
                    nc.tensor.matmul(
                        pv[:, hf * 512:(hf + 1) * 512],
                        Vaug[:, vb:vb + DH + 1],
                        ex[:, hf * 512:(hf + 1) * 512],
                        start=(kt == 0), stop=(kt == NKB - 1),
                    )

            # fin part A: free the PV accumulator fast + reciprocal of Z
            def emit_fin_a(b, h, qc):
                pv = unit_pv.pop((b, h, qc))
                a65 = apool.tile([65, QC], F32R, tag="a65")
                nc.vector.tensor_copy(a65[:], pv[0:65, :])
                rz = rpool.tile([1, QC], F32R, tag="rz")
                with nc.allow_low_precision(reason="f32r output is 32-bit"):
                    nc.vector.reciprocal(rz[:], a65[64:65, :])
                return (b, h, qc, a65, rz)

            # fin part B (emitted ~2 kt-iters later): broadcast 1/Z across
            # 64 partitions via ones-matmul, then attnN = attnU * (1/Z)
            def emit_fin_b(st):
                b, h, qc, a65, rz = st
                for half in range(QC // 512):
                    zb = psA.tile([128, TCH], F32, tag="pA")
                    nc.tensor.matmul(
                        zb[0:DH, :], ones_t[0:1, :],
                        rz[0:1, half * 512:(half + 1) * 512],
                        start=True, stop=True,
                    )
                    dst = attnN[b][h * 64:(h + 1) * 64,
                                   qc * QC + half * 512:
                                   qc * QC + (half + 1) * 512]
                    nc.vector.tensor_tensor(
                        dst, a65[0:DH, half * 512:(half + 1) * 512],
                        zb[0:DH, :], op=ALU.mult,
                    )

            # ---- output projection unit: 128 tokens x 1024, both heads ----
            out_tiles = {}

            def emit_cunit(b, rc, oc, tail=False):
                rsl = slice(b * L + rc * 128, b * L + (rc + 1) * 128)
                lsl = slice(rc * 128, (rc + 1) * 128)
                ps = psA.tile([128, TCH], F32, tag="pA")
                nc.tensor.matmul(ps[:], attnN[b][:, lsl],
                                 wo_t[:, oc * 512:(oc + 1) * 512],
                                 start=True, stop=True)
                if oc == 0:
                    out_tiles[(b, rc)] = opool.tile([128, D_MODEL], BF16,
                                                    tag="ot", name="ot")
                ot = out_tiles[(b, rc)]
                osl = slice(oc * 512, (oc + 1) * 512)
                if tail and oc == 1:
                    # after the last exp ACT is free: split tail evacuations
                    nc.scalar.activation(ot[:, osl], ps[:], AF.Copy)
                else:
                    nc.vector.tensor_copy(ot[:, osl], ps[:])
                if oc == 1:
                    nc.sync.dma_start(out_d[rsl, :], out_tiles.pop((b, rc))[:])

            # ---- schedule assembly ----
            # units ordered so c_units unlock early: per batch h0qc0, h1qc0,
            # h0qc1, h1qc1 -> tokens [0:1024] of both heads done after unit 2.
            units = [(b, h, qc) for b in range(B) for qc in range(NQC)
                     for h in range(2)]
            # proj chunks 2..7 injected into units 0..2 (after kt 3 / kt 9);
            # chunk placement respects key availability for unit 0.
            chunk_sched = {0: {4: 2, 8: 3}, 1: {4: 4, 10: 5}, 2: {4: 6, 10: 7}}
            # c_units rc-blocks injected per unit index (8 c_units each over
            # 16 kt-iters), unlocked by fin of two units prior.
            cunit_sched = {
                3: [(0, rc) for rc in range(0, 4)],
                4: [(0, rc) for rc in range(4, 8)],
                5: [(0, rc) for rc in range(8, 12)],
                6: [(0, rc) for rc in range(12, 16)],
                7: [(1, rc) for rc in range(0, 4)],
            }

            emit_warmup()
            emit_chunk(0)
            emit_chunk(1)
            pending_fin = None
            for ui, (b, h, qc) in enumerate(units):
                cq = [(cb, rc, oc) for (cb, rc) in cunit_sched.get(ui, [])
                      for oc in range(2)]
                for kt in range(NKB):
                    emit_kt(b, h, qc, kt)
                    if kt == 1 and pending_fin is not None:
                        emit_fin_b(pending_fin)
                        pending_fin = None
                    if kt in chunk_sched.get(ui, {}):
                        emit_chunk(chunk_sched[ui][kt])
                    if kt % 2 == 1 and cq:
                        emit_cunit(*cq.pop(0))
                assert not cq
                pending_fin = emit_fin_a(b, h, qc)
            emit_fin_b(pending_fin)
            # tail: last quarter of batch-1 output rows
            for rc in range(4, 16):
                for oc in range(2):
                    emit_cunit(1, rc, oc, tail=True)

    nc.compile()
    _NC_CACHE["nc"] = nc
    return nc


def _shard_inputs(x, W_qkv, b_qkv, W_o):
    import ml_dtypes
    BF = ml_dtypes.bfloat16
    xT = np.ascontiguousarray(
        x.reshape(BL, D_MODEL).T, dtype=np.float32
    ).astype(BF)
    ident = np.eye(128, dtype=np.float32).astype(BF)

    def lhsT_layout(w):
        # [D_MODEL, 128] -> [128, NKT*128] with [p, kt*128+ch] = w[kt*128+p, ch]
        return np.ascontiguousarray(
            w.reshape(NKT, 128, 128).transpose(1, 0, 2).reshape(128, NKT * 128),
            dtype=np.float32,
        ).astype(BF)

    in_maps = []
    for c in range(NCORES):
        cs = slice(c * 128, (c + 1) * 128)
        wq = W_qkv[:, cs] * 0.125
        wk = W_qkv[:, D_MODEL:][:, cs]
        wv = W_qkv[:, 2 * D_MODEL:][:, cs]
        in_maps.append({
            "xT": xT,
            "wq": lhsT_layout(wq), "wk": lhsT_layout(wk), "wv": lhsT_layout(wv),
            "bq": np.ascontiguousarray(
                b_qkv[cs] * 0.125, dtype=np.float32).reshape(128, 1),
            "bk": np.ascontiguousarray(
                b_qkv[D_MODEL:][cs], dtype=np.float32).reshape(128, 1),
            "bv": np.ascontiguousarray(
                b_qkv[2 * D_MODEL:][cs], dtype=np.float32).reshape(128, 1),
            "wo": np.ascontiguousarray(
                W_o[cs, :], dtype=np.float32).astype(BF),
            "ident": ident,
        })
    return in_maps


def _run(inputs, trace=False, tmpdir=None):
    from concourse.bass_utils import run_bass_kernel_spmd

    _register_ntff_hook()
    nc = _build()
    in_maps = _shard_inputs(
        np.asarray(inputs["x"], dtype=np.float32),
        np.asarray(inputs["W_qkv"], dtype=np.float32),
        np.asarray(inputs["b_qkv"], dtype=np.float32),
        np.asarray(inputs["W_o"], dtype=np.float32),
    )
    res = run_bass_kernel_spmd(nc, in_maps, core_ids=list(range(NCORES)),
                               trace=trace, tmpdir=tmpdir)
    partial = np.zeros((BL, D_MODEL), dtype=np.float64)
    for c in range(NCORES):
        partial += np.asarray(res.results[c]["out"]).astype(np.float64)
    out = (partial + np.asarray(inputs["b_o"], dtype=np.float64)).astype(np.float32)
    return out.reshape(B, L, D_MODEL), res


def kernel(**inputs) -> np.ndarray:
    out, _ = _run(inputs, trace=False)
    return out
